# revision 38
# baseline (speedup 1.0000x reference)
"""Trainium2 Bass kernel for nn_MultiHeadAttentionQuantum.

Reference computation (per batch element b, batch-parallel over 8 cores):
    q[s, h, w]  = x[s, :] split into 16 heads x 8 wires
    c           = cos(q + theta[w])
    out[s,h,0]  = prod(c[s,h,1:8]);  out[s,h,w>=1] = cumprod(c)[s,h,w]
    O = out merged to [S=2048, E=128]
    scores = O @ O.T / sqrt(8)       (symmetric)
    y      = softmax(scores) @ O

Device design per core (one batch element each):
  - layout [128 partitions = s%128, free = (n=s//128, e)]
  - cos via ACT Sin table (domain |arg|<4) after magic-number range
    reduction computed in "turns": r' = x/2pi + (theta+pi/2)/2pi,
    k = round(r') via +/-MAGIC, c = sin(2pi*(r'-k)).
  - segmented cumprod via 13 strided DVE multiplies.
  - scores: single fp32r (TF32) matmul — measured y-error 3.7e-4, full
    rate (1 cyc/row at N=512). exp() softmax dominates the middle phase
    anyway, so extra matmul precision terms would not be free.
  - softmax without row-max: scores <= 128/sqrt(8) = C; exp(s/sqrt(8)-C);
    row sums fused into exp via ACT accum_out.
  - attn @ O: E = exp(scores) is symmetric, so E row-blocks double as
    the column blocks of the second matmul's moving operand.
  - the quantum stage runs in two sequence-halves: scores+exp for the
    first 8 row-blocks (columns of half 1) start while half 2 is still
    in the DVE pipeline.
  - yT [e, s] chunks are PE-transposed back and scaled by 1/rowsum.
"""

import math
from contextlib import ExitStack

import numpy as np

import concourse.bass as bass
import concourse.tile as tile
from concourse import bacc, mybir
from concourse.bass_utils import run_bass_kernel_spmd
from concourse.masks import make_identity

B = 8          # batch -> one per core
S = 2048       # sequence length
E = 128        # embed dim
NB = S // 128  # 16 row blocks
W = 8          # wires per head
NH = E // W    # 16 heads
HS = S // 2    # half the sequence

F32 = mybir.dt.float32
F32R = mybir.dt.float32r

TWO_PI = float(2 * np.pi)
INV_TWO_PI = float(1 / (2 * np.pi))
MAGIC = float(1.5 * 2**23)          # fp32 round-to-nearest-int trick
HALF_PI = float(np.pi / 2)
INV_SQRT8 = float(1 / math.sqrt(8))
SCORE_MAX = float(E / math.sqrt(8))  # upper bound on any score


def build_kernel(n_cores: int = B):
    nc = bacc.Bacc(
        trn_type="TRN2", target_bir_lowering=False, debug=False,
        num_devices=n_cores,
    )
    x = nc.dram_tensor("x", [S, E], F32, kind="ExternalInput")
    theta = nc.dram_tensor("theta", [E], F32, kind="ExternalInput")
    y = nc.dram_tensor("y", [S, E], F32, kind="ExternalOutput")

    with tile.TileContext(nc) as tc, ExitStack() as ctx:
        pq = [ctx.enter_context(tc.tile_pool(name=f"pq{h}", bufs=4))
              for h in range(2)]
        ph = ctx.enter_context(tc.tile_pool(name="ph", bufs=1))
        pht = ctx.enter_context(tc.tile_pool(name="pht", bufs=4))
        pE = ctx.enter_context(tc.tile_pool(name="pE", bufs=NB))
        psmall = ctx.enter_context(tc.tile_pool(name="psmall", bufs=1))
        pstage = ctx.enter_context(tc.tile_pool(name="pstage", bufs=1))
        py = ctx.enter_context(tc.tile_pool(name="py", bufs=6))
        pscore = ctx.enter_context(tc.tile_pool(name="pscore", bufs=2, space="PSUM"))
        pout2 = ctx.enter_context(tc.tile_pool(name="pout2", bufs=3, space="PSUM"))
        pfin = ctx.enter_context(tc.tile_pool(name="pfin", bufs=1, space="PSUM"))

        # ---- input DMAs first: x halves in [p, (n, e)] layout with
        # s = p*16 + n (4KB contiguous per partition per half). The whole
        # kernel runs in this internal s-permutation (attention is
        # permutation-equivariant); the y stores undo it.
        x_v = x.ap().rearrange("(p n) e -> p n e", n=NB)
        xts = []
        dma_engs = [nc.sync, nc.scalar, nc.gpsimd, nc.gpsimd]
        for h in range(2):
            xt = pq[h].tile([128, HS], F32, tag=f"big{h}", name=f"xt{h}")
            for q in range(2):
                nb0 = (NB // 2) * h + (NB // 4) * q
                dma_engs[2 * h + q].dma_start(
                    out=xt.rearrange("p (n e) -> p n e", e=E)[:, (NB // 4) * q:
                                                              (NB // 4) * (q + 1), :],
                    in_=x_v[:, nb0:nb0 + NB // 4, :],
                )
            xts.append(xt)
        th = psmall.tile([128, E], F32)
        th_src = theta.ap()
        nc.sync.dma_start(
            out=th,
            in_=bass.AP(tensor=th_src.tensor, offset=th_src.offset,
                        ap=[[0, 128]] + list(th_src.ap)),
        )

        ident = psmall.tile([128, 128], F32)
        make_identity(nc, ident)

        # theta'' = (theta + pi/2) / 2pi, broadcast to 128 partitions
        th2 = psmall.tile([128, E], F32)
        nc.vector.tensor_scalar(
            out=th2, in0=th, scalar1=HALF_PI, scalar2=INV_TWO_PI,
            op0=mybir.AluOpType.add, op1=mybir.AluOpType.mult,
        )
        th2_b = bass.AP(tensor=th2.tensor, offset=th2.offset,
                        ap=[list(th2.ap[0]), [0, NB // 4], list(th2.ap[1])])

        neg_cmax = psmall.tile([128, 1], F32)
        nc.vector.memset(neg_cmax, -SCORE_MAX)

        H = ph.tile([128, S], F32R)
        HTc = [pht.tile([128, 512], F32R, tag=f"ht{c}", name=f"ht{c}")
               for c in range(4)]
        r_all = psmall.tile([128, 2 * NB], F32)
        E_tiles = [None] * NB

        def quantum_half(h):
            """Quantum measurement for sequence half h -> H cols, HTc[2h:2h+2]."""
            xt = xts[h]
            rp = pq[h].tile([128, HS], F32, tag=f"big{h}", name=f"rp{h}")
            ym = pq[h].tile([128, HS], F32, tag=f"big{h}", name=f"ym{h}")
            k2 = pq[h].tile([128, HS], F32, tag=f"big{h}", name=f"k2{h}")
            c = pq[h].tile([128, HS], F32, tag=f"big{h}", name=f"c{h}")
            # quarter-granular so work starts as each DMA quarter lands
            for q in range(2):
                sl = slice(q * 512, (q + 1) * 512)
                # r' = x/2pi + theta''   (angle in turns)
                nc.vector.scalar_tensor_tensor(
                    out=rp[:, sl].rearrange("p (n e) -> p n e", e=E),
                    in0=xt[:, sl].rearrange("p (n e) -> p n e", e=E),
                    scalar=INV_TWO_PI, in1=th2_b,
                    op0=mybir.AluOpType.mult, op1=mybir.AluOpType.add,
                )
                # k = round(r') by the +-MAGIC ping-pong
                nc.vector.tensor_scalar(
                    out=ym[:, sl], in0=rp[:, sl], scalar1=1.0, scalar2=MAGIC,
                    op0=mybir.AluOpType.mult, op1=mybir.AluOpType.add,
                )
                nc.vector.tensor_scalar(
                    out=k2[:, sl], in0=ym[:, sl], scalar1=MAGIC, scalar2=1.0,
                    op0=mybir.AluOpType.subtract, op1=mybir.AluOpType.mult,
                )
                # d = r' - k in [-0.5, 0.5] (in place), c = sin(2pi * d)
                nc.vector.tensor_sub(out=rp[:, sl], in0=rp[:, sl], in1=k2[:, sl])
                nc.scalar.activation(out=c[:, sl], in_=rp[:, sl],
                                     func=mybir.ActivationFunctionType.Sin,
                                     scale=TWO_PI)
            # segmented cumprod over wires within each head; the cumprod
            # chain runs on DVE, the w=0 (suffix product) chain on the
            # otherwise-idle GpSimd so the two serial chains overlap
            c4 = c.rearrange("p (n h w) -> p n h w", h=NH, w=W)
            O = pq[h].tile([128, HS], F32, tag=f"big{h}", name=f"O{h}")
            O4 = O.rearrange("p (n h w) -> p n h w", h=NH, w=W)
            nc.vector.tensor_mul(out=O4[:, :, :, 1], in0=c4[:, :, :, 0],
                                 in1=c4[:, :, :, 1])
            for w in range(2, W):
                nc.vector.tensor_mul(out=O4[:, :, :, w], in0=O4[:, :, :, w - 1],
                                     in1=c4[:, :, :, w])
            nc.vector.tensor_mul(out=O4[:, :, :, 0], in0=c4[:, :, :, 1],
                                 in1=c4[:, :, :, 2])
            for w in range(3, W):
                nc.vector.tensor_mul(out=O4[:, :, :, 0], in0=O4[:, :, :, 0],
                                     in1=c4[:, :, :, w])
            # natural-layout fp32r copy (lhsT of the attn@O matmul) and the
            # [e, s] transpose groups. For half 0 the casts run on ACT
            # (idle before the first exp); for half 1 on DVE (idle during
            # the exp phase) so they don't stretch the exp stream.
            nc.vector.tensor_copy(out=H[:, h * HS:(h + 1) * HS], in_=O)
            for g in range(2):
                pg = pout2.tile([128, 512], F32, tag="po", name=f"ptr{h}{g}")
                for b in range(4):
                    nc.tensor.transpose(
                        out=pg[:, b * 128:(b + 1) * 128],
                        in_=O[:, (g * 4 + b) * 128:(g * 4 + b) * 128 + 128],
                        identity=ident,
                    )
                nc.vector.tensor_copy(out=HTc[2 * h + g], in_=pg)

        def scores_block(i, hf):
            """scores block-row i, half hf -> E_tiles[i][:, hf half] + rowsum.

            Partial row sums: the hf=1 half uses the exp's fused accum_out;
            the hf=0 half is reduced on DVE (which has slack during the exp
            phase) to keep the saturated ACT stream free of accumulator
            drains."""
            ps = pscore.tile([128, 1024], F32, tag="ps", name="ps")
            for cc in range(2):
                nc.tensor.matmul(
                    out=ps[:, cc * 512:(cc + 1) * 512],
                    lhsT=HTc[i // 4][:, (i % 4) * 128:(i % 4) * 128 + 128],
                    rhs=HTc[hf * 2 + cc],
                    start=True, stop=True,
                )
            Eslice = E_tiles[i][:, hf * 1024:(hf + 1) * 1024]
            if hf == 0:
                nc.scalar.activation(
                    out=Eslice, in_=ps,
                    func=mybir.ActivationFunctionType.Exp,
                    bias=neg_cmax, scale=INV_SQRT8,
                )
                nc.vector.tensor_reduce(
                    out=r_all[:, 2 * i:2 * i + 1],
                    in_=Eslice.bitcast(F32),
                    axis=mybir.AxisListType.X, op=mybir.AluOpType.add,
                )
            else:
                nc.scalar.activation(
                    out=Eslice, in_=ps,
                    func=mybir.ActivationFunctionType.Exp,
                    bias=neg_cmax, scale=INV_SQRT8,
                    accum_out=r_all[:, 2 * i + 1:2 * i + 2],
                )

        # ---- half 1 quantum, then the score work that only needs half 1
        quantum_half(0)
        for i in range(NB // 2):
            E_tiles[i] = pE.tile([128, S], F32R, tag="Ei", name=f"E{i}")
            scores_block(i, 0)
        # ---- half 2 quantum (DVE/ACT pipeline overlaps the above PE work)
        quantum_half(1)
        for i in range(NB // 2, NB):
            E_tiles[i] = pE.tile([128, S], F32R, tag="Ei", name=f"E{i}")
            scores_block(i, 0)
        # hf=1 blocks; each row's softmax denominator follows immediately so
        # the output pipeline is gated only by the last block's exp
        recip = psmall.tile([128, NB], F32)
        ra = r_all.rearrange("p (i two) -> p i two", two=2)
        for i in range(NB):
            scores_block(i, 1)
            nc.vector.tensor_add(out=recip[:, i:i + 1], in0=ra[:, i, :][:, 0:1],
                                 in1=ra[:, i, :][:, 1:2])
            nc.vector.reciprocal(out=recip[:, i:i + 1], in_=recip[:, i:i + 1])

        # ---- yT[e, s] = sum_t H[t,e] E[t,s]; transpose back, scale, store
        # internal f = si*128 + q  ->  DRAM row q*16 + si
        y_v = y.ap().rearrange("(q n) e -> n q e", n=NB)
        for j in range(4):
            po = pout2.tile([128, 512], F32, tag="po", name=f"po{j}")
            for kt in range(NB):
                nc.tensor.matmul(
                    out=po,
                    lhsT=H[:, kt * 128:(kt + 1) * 128],
                    rhs=E_tiles[kt][:, j * 512:(j + 1) * 512],
                    start=(kt == 0), stop=(kt == NB - 1),
                )
            stage = pstage.tile([128, 512], F32, tag="st", name="st")
            nc.vector.tensor_copy(out=stage, in_=po)
            pt = pfin.tile([128, 512], F32, tag="fin", name="ptb")
            for jj in range(4):
                nc.tensor.transpose(
                    out=pt[:, jj * 128:(jj + 1) * 128],
                    in_=stage[:, jj * 128:(jj + 1) * 128], identity=ident,
                )
            for jj in range(4):
                si = j * 4 + jj
                y_sb = py.tile([128, 128], F32, tag="ysb", name="ysb")
                nc.vector.tensor_scalar_mul(
                    out=y_sb, in0=pt[:, jj * 128:(jj + 1) * 128],
                    scalar1=recip[:, si:si + 1],
                )
                eng = nc.sync if si % 2 == 0 else nc.gpsimd
                eng.dma_start(out=y_v[si], in_=y_sb)

    nc.compile()
    return nc


_NC_CACHE = None


def _get_nc():
    global _NC_CACHE
    if _NC_CACHE is None:
        _NC_CACHE = build_kernel()
    return _NC_CACHE


def kernel(x: np.ndarray, theta: np.ndarray) -> np.ndarray:
    """x: [8, 2048, 128] f32, theta: [8] f32 -> [8, 2048, 128] f32."""
    assert x.shape == (B, S, E) and theta.shape == (W,)
    nc = _get_nc()
    theta_row = np.tile(np.ascontiguousarray(theta, dtype=np.float32), E // W)
    in_maps = [
        {"x": np.ascontiguousarray(x[b], dtype=np.float32), "theta": theta_row}
        for b in range(B)
    ]
    res = run_bass_kernel_spmd(nc, in_maps, core_ids=list(range(B)))
    return np.stack([res.results[b]["y"] for b in range(B)], axis=0)


# revision 39
# speedup vs baseline: 1.0591x; 1.0591x over previous
"""Trainium2 Bass kernel for nn_MultiHeadAttentionQuantum.

Reference computation (per batch element b, batch-parallel over 8 cores):
    q[s, h, w]  = x[s, :] split into 16 heads x 8 wires
    c           = cos(q + theta[w])
    out[s,h,0]  = prod(c[s,h,1:8]);  out[s,h,w>=1] = cumprod(c)[s,h,w]
    O = out merged to [S=2048, E=128]
    scores = O @ O.T / sqrt(8)       (symmetric)
    y      = softmax(scores) @ O

Device design per core (one batch element each):
  - layout [128 partitions = s%128, free = (n=s//128, e)]
  - cos via ACT Sin table (domain |arg|<4) after magic-number range
    reduction computed in "turns": r' = x/2pi + (theta+pi/2)/2pi,
    k = round(r') via +/-MAGIC, c = sin(2pi*(r'-k)).
  - segmented cumprod via 13 strided DVE multiplies.
  - scores: single fp32r (TF32) matmul — measured y-error 3.7e-4, full
    rate (1 cyc/row at N=512). exp() softmax dominates the middle phase
    anyway, so extra matmul precision terms would not be free.
  - softmax without row-max: scores <= 128/sqrt(8) = C; exp(s/sqrt(8)-C);
    row sums fused into exp via ACT accum_out.
  - attn @ O: E = exp(scores) is symmetric, so E row-blocks double as
    the column blocks of the second matmul's moving operand.
  - the quantum stage runs in two sequence-halves: scores+exp for the
    first 8 row-blocks (columns of half 1) start while half 2 is still
    in the DVE pipeline.
  - yT [e, s] chunks are PE-transposed back and scaled by 1/rowsum.
"""

import math
from contextlib import ExitStack

import numpy as np

import concourse.bass as bass
import concourse.tile as tile
from concourse import bacc, mybir
from concourse.bass_utils import run_bass_kernel_spmd
from concourse.masks import make_identity

B = 8          # batch -> one per core
S = 2048       # sequence length
E = 128        # embed dim
NB = S // 128  # 16 row blocks
W = 8          # wires per head
NH = E // W    # 16 heads
HS = S // 2    # half the sequence

F32 = mybir.dt.float32
F32R = mybir.dt.float32r

TWO_PI = float(2 * np.pi)
INV_TWO_PI = float(1 / (2 * np.pi))
MAGIC = float(1.5 * 2**23)          # fp32 round-to-nearest-int trick
HALF_PI = float(np.pi / 2)
INV_SQRT8 = float(1 / math.sqrt(8))
SCORE_MAX = float(E / math.sqrt(8))  # upper bound on any score


def build_kernel(n_cores: int = B):
    nc = bacc.Bacc(
        trn_type="TRN2", target_bir_lowering=False, debug=False,
        num_devices=n_cores,
    )
    x = nc.dram_tensor("x", [S, E], F32, kind="ExternalInput")
    theta = nc.dram_tensor("theta", [E], F32, kind="ExternalInput")
    y = nc.dram_tensor("y", [S, E], F32, kind="ExternalOutput")

    with tile.TileContext(nc) as tc, ExitStack() as ctx:
        pq = [ctx.enter_context(tc.tile_pool(name=f"pq{h}", bufs=4))
              for h in range(2)]
        ph = ctx.enter_context(tc.tile_pool(name="ph", bufs=1))
        pht = ctx.enter_context(tc.tile_pool(name="pht", bufs=4))
        pE = ctx.enter_context(tc.tile_pool(name="pE", bufs=NB))
        psmall = ctx.enter_context(tc.tile_pool(name="psmall", bufs=1))
        pstage = ctx.enter_context(tc.tile_pool(name="pstage", bufs=1))
        py = ctx.enter_context(tc.tile_pool(name="py", bufs=6))
        pscore = ctx.enter_context(tc.tile_pool(name="pscore", bufs=2, space="PSUM"))
        pout2 = ctx.enter_context(tc.tile_pool(name="pout2", bufs=3, space="PSUM"))
        pfin = ctx.enter_context(tc.tile_pool(name="pfin", bufs=1, space="PSUM"))

        # ---- input DMAs first: x halves in [p, (n, e)] layout with
        # s = p*16 + n (4KB contiguous per partition per half). The whole
        # kernel runs in this internal s-permutation (attention is
        # permutation-equivariant); the y stores undo it.
        x_v = x.ap().rearrange("(p n) e -> p n e", n=NB)
        xts = []
        dma_engs = [nc.gpsimd, nc.scalar, nc.sync, nc.gpsimd]
        for h in range(2):
            xt = pq[h].tile([128, HS], F32, tag=f"big{h}", name=f"xt{h}")
            for q in range(2):
                nb0 = (NB // 2) * h + (NB // 4) * q
                dma_engs[2 * h + q].dma_start(
                    out=xt.rearrange("p (n e) -> p n e", e=E)[:, (NB // 4) * q:
                                                              (NB // 4) * (q + 1), :],
                    in_=x_v[:, nb0:nb0 + NB // 4, :],
                )
            xts.append(xt)
        th = psmall.tile([128, E], F32)
        th_src = theta.ap()
        nc.sync.dma_start(
            out=th,
            in_=bass.AP(tensor=th_src.tensor, offset=th_src.offset,
                        ap=[[0, 128]] + list(th_src.ap)),
        )

        ident = psmall.tile([128, 128], F32)
        make_identity(nc, ident)

        # theta'' = (theta + pi/2) / 2pi, broadcast to 128 partitions
        th2 = psmall.tile([128, E], F32)
        nc.vector.tensor_scalar(
            out=th2, in0=th, scalar1=HALF_PI, scalar2=INV_TWO_PI,
            op0=mybir.AluOpType.add, op1=mybir.AluOpType.mult,
        )
        th2_b = bass.AP(tensor=th2.tensor, offset=th2.offset,
                        ap=[list(th2.ap[0]), [0, NB // 4], list(th2.ap[1])])

        neg_cmax = psmall.tile([128, 1], F32)
        nc.vector.memset(neg_cmax, -SCORE_MAX)

        H = ph.tile([128, S], F32R)
        HTc = [pht.tile([128, 512], F32R, tag=f"ht{c}", name=f"ht{c}")
               for c in range(4)]
        r_all = psmall.tile([128, 2 * NB], F32)
        E_tiles = [None] * NB

        def quantum_half(h):
            """Quantum measurement for sequence half h -> H cols, HTc[2h:2h+2]."""
            xt = xts[h]
            rp = pq[h].tile([128, HS], F32, tag=f"big{h}", name=f"rp{h}")
            ym = pq[h].tile([128, HS], F32, tag=f"big{h}", name=f"ym{h}")
            k2 = pq[h].tile([128, HS], F32, tag=f"big{h}", name=f"k2{h}")
            c = pq[h].tile([128, HS], F32, tag=f"big{h}", name=f"c{h}")
            # quarter-granular so work starts as each DMA quarter lands
            for q in range(2):
                sl = slice(q * 512, (q + 1) * 512)
                # r' = x/2pi + theta''   (angle in turns)
                nc.vector.scalar_tensor_tensor(
                    out=rp[:, sl].rearrange("p (n e) -> p n e", e=E),
                    in0=xt[:, sl].rearrange("p (n e) -> p n e", e=E),
                    scalar=INV_TWO_PI, in1=th2_b,
                    op0=mybir.AluOpType.mult, op1=mybir.AluOpType.add,
                )
                # k = round(r') by the +-MAGIC ping-pong
                nc.vector.tensor_scalar(
                    out=ym[:, sl], in0=rp[:, sl], scalar1=1.0, scalar2=MAGIC,
                    op0=mybir.AluOpType.mult, op1=mybir.AluOpType.add,
                )
                nc.vector.tensor_scalar(
                    out=k2[:, sl], in0=ym[:, sl], scalar1=MAGIC, scalar2=1.0,
                    op0=mybir.AluOpType.subtract, op1=mybir.AluOpType.mult,
                )
                # d = r' - k in [-0.5, 0.5] (in place), c = sin(2pi * d)
                nc.vector.tensor_sub(out=rp[:, sl], in0=rp[:, sl], in1=k2[:, sl])
                nc.scalar.activation(out=c[:, sl], in_=rp[:, sl],
                                     func=mybir.ActivationFunctionType.Sin,
                                     scale=TWO_PI)
            # segmented cumprod over wires within each head
            c4 = c.rearrange("p (n h w) -> p n h w", h=NH, w=W)
            O = pq[h].tile([128, HS], F32, tag=f"big{h}", name=f"O{h}")
            O4 = O.rearrange("p (n h w) -> p n h w", h=NH, w=W)
            nc.vector.tensor_mul(out=O4[:, :, :, 1], in0=c4[:, :, :, 0],
                                 in1=c4[:, :, :, 1])
            for w in range(2, W):
                nc.vector.tensor_mul(out=O4[:, :, :, w], in0=O4[:, :, :, w - 1],
                                     in1=c4[:, :, :, w])
            nc.vector.tensor_mul(out=O4[:, :, :, 0], in0=c4[:, :, :, 1],
                                 in1=c4[:, :, :, 2])
            for w in range(3, W):
                nc.vector.tensor_mul(out=O4[:, :, :, 0], in0=O4[:, :, :, 0],
                                     in1=c4[:, :, :, w])
            # natural-layout fp32r copy (lhsT of the attn@O matmul)
            nc.vector.tensor_copy(out=H[:, h * HS:(h + 1) * HS], in_=O)
            # transpose to [e, s] in two 512-col groups, cast to fp32r
            for g in range(2):
                pg = pout2.tile([128, 512], F32, tag="po", name=f"ptr{h}{g}")
                for b in range(4):
                    nc.tensor.transpose(
                        out=pg[:, b * 128:(b + 1) * 128],
                        in_=O[:, (g * 4 + b) * 128:(g * 4 + b) * 128 + 128],
                        identity=ident,
                    )
                nc.vector.tensor_copy(out=HTc[2 * h + g], in_=pg)

        def scores_block(i, hf):
            """scores block-row i, half hf -> E_tiles[i][:, hf half] + rowsum."""
            ps = pscore.tile([128, 1024], F32, tag="ps", name="ps")
            for cc in range(2):
                nc.tensor.matmul(
                    out=ps[:, cc * 512:(cc + 1) * 512],
                    lhsT=HTc[i // 4][:, (i % 4) * 128:(i % 4) * 128 + 128],
                    rhs=HTc[hf * 2 + cc],
                    start=True, stop=True,
                )
            nc.scalar.activation(
                out=E_tiles[i][:, hf * 1024:(hf + 1) * 1024], in_=ps,
                func=mybir.ActivationFunctionType.Exp,
                bias=neg_cmax, scale=INV_SQRT8,
                accum_out=r_all[:, 2 * i + hf:2 * i + hf + 1],
            )

        # ---- half 1 quantum, then the score work that only needs half 1
        quantum_half(0)
        for i in range(NB // 2):
            E_tiles[i] = pE.tile([128, S], F32R, tag="Ei", name=f"E{i}")
            scores_block(i, 0)
        # ---- half 2 quantum (DVE/ACT pipeline overlaps the above PE work)
        quantum_half(1)
        for i in range(NB // 2, NB):
            E_tiles[i] = pE.tile([128, S], F32R, tag="Ei", name=f"E{i}")
            scores_block(i, 0)
        # hf=1 blocks; each row's softmax denominator follows immediately so
        # the output pipeline is gated only by the last block's exp
        recip = psmall.tile([128, NB], F32)
        ra = r_all.rearrange("p (i two) -> p i two", two=2)
        for i in range(NB):
            scores_block(i, 1)
            nc.vector.tensor_add(out=recip[:, i:i + 1], in0=ra[:, i, :][:, 0:1],
                                 in1=ra[:, i, :][:, 1:2])
            nc.vector.reciprocal(out=recip[:, i:i + 1], in_=recip[:, i:i + 1])

        # ---- yT[e, s] = sum_t H[t,e] E[t,s]; transpose back, scale, store
        # internal f = si*128 + q  ->  DRAM row q*16 + si
        y_v = y.ap().rearrange("(q n) e -> n q e", n=NB)
        for j in range(4):
            po = pout2.tile([128, 512], F32, tag="po", name=f"po{j}")
            for kt in range(NB):
                nc.tensor.matmul(
                    out=po,
                    lhsT=H[:, kt * 128:(kt + 1) * 128],
                    rhs=E_tiles[kt][:, j * 512:(j + 1) * 512],
                    start=(kt == 0), stop=(kt == NB - 1),
                )
            stage = pstage.tile([128, 512], F32, tag="st", name="st")
            nc.vector.tensor_copy(out=stage, in_=po)
            pt = pfin.tile([128, 512], F32, tag="fin", name="ptb")
            for jj in range(4):
                nc.tensor.transpose(
                    out=pt[:, jj * 128:(jj + 1) * 128],
                    in_=stage[:, jj * 128:(jj + 1) * 128], identity=ident,
                )
            for jj in range(4):
                si = j * 4 + jj
                y_sb = py.tile([128, 128], F32, tag="ysb", name="ysb")
                nc.vector.tensor_scalar_mul(
                    out=y_sb, in0=pt[:, jj * 128:(jj + 1) * 128],
                    scalar1=recip[:, si:si + 1],
                )
                eng = nc.sync if si % 2 == 0 else nc.gpsimd
                eng.dma_start(out=y_v[si], in_=y_sb)

    nc.compile()
    return nc


_NC_CACHE = None


def _get_nc():
    global _NC_CACHE
    if _NC_CACHE is None:
        _NC_CACHE = build_kernel()
    return _NC_CACHE


def kernel(x: np.ndarray, theta: np.ndarray) -> np.ndarray:
    """x: [8, 2048, 128] f32, theta: [8] f32 -> [8, 2048, 128] f32."""
    assert x.shape == (B, S, E) and theta.shape == (W,)
    nc = _get_nc()
    theta_row = np.tile(np.ascontiguousarray(theta, dtype=np.float32), E // W)
    in_maps = [
        {"x": np.ascontiguousarray(x[b], dtype=np.float32), "theta": theta_row}
        for b in range(B)
    ]
    res = run_bass_kernel_spmd(nc, in_maps, core_ids=list(range(B)))
    return np.stack([res.results[b]["y"] for b in range(B)], axis=0)
